# revision 11
# baseline (speedup 1.0000x reference)
"""Trainium2 Bass kernel for ChannelIndependentConv (mode=1) message passing.

Contract: kernel(**inputs) takes the FULL unsharded inputs (batch B=8) and
returns (node_out, edge_out) matching the reference. Internally the batch is
sharded 1-per-core across 8 NeuronCores; the small Linear weights are
replicated (data-parallel over batch).

Math per batch b:
    node_x  = emb_node @ Wn + bn                      [N, D]
    node_sx = emb_node @ Wsn + bsn                    [N, D]
    edge_x  = emb_edge @ We + be                      [N, N, D]
    agg[i,c] = sum_j A[i,j] * edge_x[i,j,c] * node_x[j,c]
    node_out = relu(agg) + relu(node_sx)
    edge_out = relu(edge_x)

Device pipeline (per core, N=256, D=64), natural token layout [128 j', ...]:
  - A-mask multiply on GPSIMD with a free-dim broadcast AP.
  - agg via one N=512 matmul per bank with node_x as the stationary operand
    (output lands transposed as HT[c, (i, k)]), then a DVE multiply by We^T
    and a segmented free-dim reduce.
  - edge_x via PE transposes of the input tiles + one block-diag(We, We)
    matmul per bank; relu+bias on ACT during PSUM eviction; PE transposes
    back to natural layout for contiguous HBM stores.
"""

import os
import numpy as np

import concourse.bacc as bacc
import concourse.mybir as mybir
from concourse.tile import TileContext
from concourse.bass import broadcast_tensor_aps
from concourse.bass_utils import run_bass_kernel_spmd

B, N, D = 8, 256, 64
NCORES = 8
NG = N // 8  # 32 groups of 8 i-rows
F32 = mybir.dt.float32
F32R = mybir.dt.float32r

# f32r (BASSK_F32R=1): fast-PE subset — f32r only for the H/main/S_A
# matmuls (~12-bit mantissa rounding at the input DMAs, rel err ~2e-4);
# all transposes stay on the exact fp32 path (the all-f32r config, with
# f32r transpose-mode, hard-crashed the core). Default is exact fp32.
USE_F32R = os.environ.get("BASSK_F32R", "0") == "1"

_CACHE = {}


def _build_nc(use_f32r: bool):
    nc = bacc.Bacc("TRN2", target_bir_lowering=False, debug=False,
                   num_devices=NCORES)
    mmdt = F32R if use_f32r else F32

    def mm(ap):
        return ap.bitcast(F32R) if use_f32r else ap

    emb = nc.dram_tensor("emb", [N, N, D], F32, kind="ExternalInput").ap()
    at_d = nc.dram_tensor("at", [N, N], F32, kind="ExternalInput").ap()
    y_d = nc.dram_tensor("y", [N, D], F32, kind="ExternalInput").ap()
    rnsx_d = nc.dram_tensor("rnsx", [N, D], F32, kind="ExternalInput").ap()
    wbd_d = nc.dram_tensor("wbd", [128, 128], F32, kind="ExternalInput").ap()
    wet_d = nc.dram_tensor("wet", [D, D], F32, kind="ExternalInput").ap()
    be2_d = nc.dram_tensor("be2", [128, 1], F32, kind="ExternalInput").ap()
    berep_d = nc.dram_tensor("berep", [128, D], F32, kind="ExternalInput").ap()
    ident_d = nc.dram_tensor("ident", [128, 128], F32, kind="ExternalInput").ap()
    edge_out = nc.dram_tensor("edge_out", [N, N, D], F32,
                              kind="ExternalOutput").ap()
    node_out = nc.dram_tensor("node_out", [N, D], F32,
                              kind="ExternalOutput").ap()

    with TileContext(nc) as tc:
        with tc.tile_pool(name="consts", bufs=1) as cp, \
             tc.tile_pool(name="work", bufs=4) as wp, \
             tc.tile_pool(name="psum", bufs=2, space="PSUM") as pp:
            # ---- persistent constants ----
            at_sb = [cp.tile([128, N], mmdt, tag=f"at{h}", name=f"at{h}") for h in range(2)]
            y_sb = [cp.tile([128, D], mmdt, tag=f"y{h}", name=f"y{h}") for h in range(2)]
            rnsx_sb = [cp.tile([128, D], F32, tag=f"rn{h}", name=f"rn{h}") for h in range(2)]
            wbd_sb = cp.tile([128, 128], mmdt, tag="wbd")
            wet_sb = cp.tile([D, D], F32, tag="wet")
            be2_sb = cp.tile([128, 1], F32, tag="be2")
            berep_sb = cp.tile([128, D], F32, tag="berep")
            id_sb = cp.tile([128, 128], F32, tag="ident")
            aggt_sb = cp.tile([D, N], F32, tag="aggt")
            besa_sb = [cp.tile([128, D], F32, tag=f"besa{h}", name=f"besa{h}") for h in range(2)]

            for h in range(2):
                nc.sync.dma_start(out=at_sb[h][:, :],
                                  in_=mm(at_d[128 * h:128 * h + 128, :]))
                nc.sync.dma_start(out=y_sb[h][:, :],
                                  in_=mm(y_d[128 * h:128 * h + 128, :]))
                nc.sync.dma_start(out=rnsx_sb[h][:, :],
                                  in_=rnsx_d[128 * h:128 * h + 128, :])
            nc.sync.dma_start(out=wbd_sb[:, :], in_=mm(wbd_d))
            nc.sync.dma_start(out=wet_sb[:, :], in_=wet_d)
            nc.sync.dma_start(out=be2_sb[:, :], in_=be2_d)
            nc.sync.dma_start(out=berep_sb[:, :], in_=berep_d)
            nc.sync.dma_start(out=id_sb[:, :], in_=ident_d)

            # ---- S_A = A @ node_x, then beSA = be * S_A  (for the bias
            # contribution to agg: sum_j A[i,j]*be[c]*Y[j,c]) ----
            for ib in range(2):
                sa_ps = pp.tile([128, D], F32, tag="pT")
                for h in range(2):
                    nc.tensor.matmul(sa_ps[:, :],
                                     at_sb[h][:, 128 * ib:128 * ib + 128],
                                     y_sb[h][:, :],
                                     start=(h == 0), stop=(h == 1))
                nc.vector.tensor_tensor(besa_sb[ib][:, :], sa_ps[:, :],
                                        berep_sb[:, :], mybir.AluOpType.mult)

            # ---- main loop over 32 groups of 8 i-rows ----
            for g in range(NG):
                i0 = 8 * g
                h_ps = pp.tile([D, 512], F32, tag="H")
                # one 512KB load per group; free layout (i, h, k) so the
                # HBM-side (i, h) dims merge into one 3-dim balanced AP
                emb2 = wp.tile([128, 1024], mmdt, tag="emb")
                emb2_r = emb2[:, :].rearrange("p (i h k) -> p i h k",
                                              i=8, h=2)
                for h in range(2):
                    nc.sync.dma_start(
                        out=emb2_r[:, :, h, :],
                        in_=mm(emb[i0:i0 + 8, 128 * h:128 * h + 128, :]
                               .rearrange("i j k -> j i k")))
                emb2_f = emb2[:, :].bitcast(F32).rearrange(
                    "p (i h k) -> p i h k", i=8, h=2)

                for h in range(2):
                    # mask: memb = emb * A^T[j', i]  (broadcast over k)
                    memb = wp.tile([128, 512], mmdt, tag="memb")
                    e_v = emb2_f[:, :, h, :]
                    a_v = at_sb[h][:, i0:i0 + 8].bitcast(F32).rearrange(
                        "p (i u) -> p i u", u=1)
                    e_b, a_b = broadcast_tensor_aps(e_v, a_v)
                    nc.gpsimd.tensor_tensor(
                        memb[:, :].bitcast(F32).rearrange(
                            "p (i k) -> p i k", i=8),
                        e_b, a_b, mybir.AluOpType.mult)
                    # H[c, (i, k)] += sum_j' Y[j', c] * memb[j', (i, k)]
                    nc.tensor.matmul(h_ps[:, :], y_sb[h][:, :], memb[:, :],
                                     start=(h == 0), stop=(h == 1))

                for q in range(2):
                    # forward transposes: [j', (h2, k)] -> [(h2, k), j'] per i
                    psumT = pp.tile([128, 512], F32, tag="pT")
                    for w in range(4):
                        isb = 4 * q + w
                        nc.tensor.matmul(psumT[:, 128 * w:128 * w + 128],
                                         emb2[:, 128 * isb:128 * isb + 128]
                                         .bitcast(F32),
                                         id_sb[:, :], is_transpose=True,
                                         start=True, stop=True)
                    embt = wp.tile([128, 512], mmdt, tag="embt")
                    nc.scalar.activation(embt[:, :], psumT[:, :],
                                         mybir.ActivationFunctionType.Copy)

                    # block-diag(We, We) main matmul -> edge_x^T in PSUM
                    e_ps = pp.tile([128, 512], F32, tag="pE")
                    nc.tensor.matmul(e_ps[:, :], wbd_sb[:, :], embt[:, :],
                                     start=True, stop=True)

                    # relu(+be) eviction (edge_out path, transposed layout)
                    eot = wp.tile([128, 512], F32, tag="eot")
                    nc.scalar.activation(eot[:, :], e_ps[:, :],
                                         mybir.ActivationFunctionType.Relu,
                                         bias=be2_sb[:, :])

                    # back transposes -> natural [j', (h, c)] per i
                    pnat = pp.tile([128, 512], F32, tag="pN")
                    for w in range(4):
                        nc.tensor.transpose(pnat[:, 128 * w:128 * w + 128],
                                            eot[:, 128 * w:128 * w + 128],
                                            id_sb[:, :].bitcast(F32))
                    eo = wp.tile([128, 512], F32, tag="eo")
                    nc.vector.tensor_copy(eo[:, :], pnat[:, :])
                    nc.scalar.dma_start(
                        out=edge_out[i0 + 4 * q:i0 + 4 * q + 4, :, :]
                            .rearrange("i (h j) c -> j i h c", h=2),
                        in_=eo[:, :].rearrange("p (i h c) -> p i h c",
                                               i=4, h=2))

                # agg: multiply H by We^T (bcast over i), reduce over k
                hw_sb = wp.tile([D, 512], F32, tag="hw")
                h_v = h_ps[:, :].rearrange("p (i k) -> p i k", i=8)
                w_v = wet_sb[:, :].rearrange("p (u k) -> p u k", u=1)
                h_b, w_b = broadcast_tensor_aps(h_v, w_v)
                nc.vector.tensor_tensor(
                    hw_sb[:, :].rearrange("p (i k) -> p i k", i=8),
                    h_b, w_b, mybir.AluOpType.mult)
                nc.vector.tensor_reduce(
                    aggt_sb[:, 8 * g:8 * g + 8],
                    hw_sb[:, :].rearrange("p (i k) -> p i k", i=8),
                    mybir.AxisListType.X, mybir.AluOpType.add)

            # ---- finalize node_out = relu(aggT^T + beSA) + relu_nsx ----
            for ib in range(2):
                agg_ps = pp.tile([128, D], F32, tag="pT")
                nc.tensor.transpose(agg_ps[:, :],
                                    aggt_sb[:, 128 * ib:128 * ib + 128],
                                    id_sb[0:D, 0:D].bitcast(F32))
                full = wp.tile([128, D], F32, tag="nfull")
                nc.vector.tensor_tensor(full[:, :], agg_ps[:, :],
                                        besa_sb[ib][:, :],
                                        mybir.AluOpType.add)
                rfull = wp.tile([128, D], F32, tag="nrelu")
                nc.scalar.activation(rfull[:, :], full[:, :],
                                     mybir.ActivationFunctionType.Relu)
                nsb = wp.tile([128, D], F32, tag="nout")
                nc.vector.tensor_tensor(nsb[:, :], rfull[:, :],
                                        rnsx_sb[ib][:, :],
                                        mybir.AluOpType.add)
                nc.sync.dma_start(out=node_out[128 * ib:128 * ib + 128, :],
                                  in_=nsb[:, :])

    nc.compile()
    return nc


def _host_prep(A, emb_node, emb_edge, Wn, bn, Wsn, bsn, We, be):
    """Per-core input maps. Heavy data (emb_edge) passes through untouched;
    only O(N*D)-scale tensors are laid out host-side."""
    A = np.asarray(A, np.float32)
    emb_node = np.asarray(emb_node, np.float32)
    emb_edge = np.ascontiguousarray(np.asarray(emb_edge, np.float32))
    Wn, bn = np.asarray(Wn, np.float32), np.asarray(bn, np.float32)
    Wsn, bsn = np.asarray(Wsn, np.float32), np.asarray(bsn, np.float32)
    We, be = np.asarray(We, np.float32), np.asarray(be, np.float32)

    wbd = np.zeros((128, 128), np.float32)
    wbd[:D, :D] = We
    wbd[D:, D:] = We
    wet = np.ascontiguousarray(We.T)
    be2 = np.concatenate([be, be])[:, None].astype(np.float32)
    berep = np.broadcast_to(be, (128, D)).astype(np.float32).copy()
    ident = np.eye(128, dtype=np.float32)

    in_maps = []
    for b in range(B):
        y = emb_node[b] @ Wn + bn
        rnsx = np.maximum(emb_node[b] @ Wsn + bsn, 0.0)
        in_maps.append({
            "emb": emb_edge[b],
            "at": np.ascontiguousarray(A[b].T),
            "y": np.ascontiguousarray(y.astype(np.float32)),
            "rnsx": np.ascontiguousarray(rnsx.astype(np.float32)),
            "wbd": wbd, "wet": wet, "be2": be2, "berep": berep,
            "ident": ident,
        })
    return in_maps


def _get_nc():
    key = ("nc", USE_F32R)
    if key not in _CACHE:
        _CACHE[key] = _build_nc(USE_F32R)
    return _CACHE[key]


def kernel(A, emb_node, emb_edge, Wn, bn, Wsn, bsn, We, be):
    in_maps = _host_prep(A, emb_node, emb_edge, Wn, bn, Wsn, bsn, We, be)
    nc = _get_nc()
    res = run_bass_kernel_spmd(nc, in_maps, core_ids=list(range(NCORES)))
    node = np.stack([res.results[b]["node_out"] for b in range(B)])
    edge = np.stack([res.results[b]["edge_out"] for b in range(B)])
    return node, edge


# revision 12
# speedup vs baseline: 1.0232x; 1.0232x over previous
"""Trainium2 Bass kernel for ChannelIndependentConv (mode=1) message passing.

Contract: kernel(**inputs) takes the FULL unsharded inputs (batch B=8) and
returns (node_out, edge_out) matching the reference. Internally the batch is
sharded 1-per-core across 8 NeuronCores; the small Linear weights are
replicated (data-parallel over batch).

Math per batch b:
    node_x  = emb_node @ Wn + bn                      [N, D]
    node_sx = emb_node @ Wsn + bsn                    [N, D]
    edge_x  = emb_edge @ We + be                      [N, N, D]
    agg[i,c] = sum_j A[i,j] * edge_x[i,j,c] * node_x[j,c]
    node_out = relu(agg) + relu(node_sx)
    edge_out = relu(edge_x)

Device pipeline (per core, N=256, D=64), natural token layout [128 j', ...]:
  - A-mask multiply on GPSIMD with a free-dim broadcast AP.
  - agg via one N=512 matmul per bank with node_x as the stationary operand
    (output lands transposed as HT[c, (i, k)]), then a DVE multiply by We^T
    and a segmented free-dim reduce.
  - edge_x via PE transposes of the input tiles + one block-diag(We, We)
    matmul per bank; relu+bias on ACT during PSUM eviction; PE transposes
    back to natural layout for contiguous HBM stores.
"""

import os
import numpy as np

import concourse.bacc as bacc
import concourse.mybir as mybir
from concourse.tile import TileContext
from concourse.bass import broadcast_tensor_aps
from concourse.bass_utils import run_bass_kernel_spmd

B, N, D = 8, 256, 64
NCORES = 8
NG = N // 8  # 32 groups of 8 i-rows
F32 = mybir.dt.float32
F32R = mybir.dt.float32r

# f32r (BASSK_F32R=1): fast-PE subset — f32r only for the H/main/S_A
# matmuls (~12-bit mantissa rounding at the input DMAs, rel err ~2e-4);
# all transposes stay on the exact fp32 path (the all-f32r config, with
# f32r transpose-mode, hard-crashed the core). Default is exact fp32.
USE_F32R = os.environ.get("BASSK_F32R", "0") == "1"

_CACHE = {}


def _build_nc(use_f32r: bool):
    nc = bacc.Bacc("TRN2", target_bir_lowering=False, debug=False,
                   num_devices=NCORES)
    mmdt = F32R if use_f32r else F32

    def mm(ap):
        return ap.bitcast(F32R) if use_f32r else ap

    emb = nc.dram_tensor("emb", [N, N, D], F32, kind="ExternalInput").ap()
    at_d = nc.dram_tensor("at", [N, N], F32, kind="ExternalInput").ap()
    y_d = nc.dram_tensor("y", [N, D], F32, kind="ExternalInput").ap()
    rnsx_d = nc.dram_tensor("rnsx", [N, D], F32, kind="ExternalInput").ap()
    wbd_d = nc.dram_tensor("wbd", [128, 128], F32, kind="ExternalInput").ap()
    wet_d = nc.dram_tensor("wet", [D, D], F32, kind="ExternalInput").ap()
    be2_d = nc.dram_tensor("be2", [128, 1], F32, kind="ExternalInput").ap()
    berep_d = nc.dram_tensor("berep", [128, D], F32, kind="ExternalInput").ap()
    ident_d = nc.dram_tensor("ident", [128, 128], F32, kind="ExternalInput").ap()
    edge_out = nc.dram_tensor("edge_out", [N, N, D], F32,
                              kind="ExternalOutput").ap()
    node_out = nc.dram_tensor("node_out", [N, D], F32,
                              kind="ExternalOutput").ap()

    with TileContext(nc) as tc:
        with tc.tile_pool(name="consts", bufs=1) as cp, \
             tc.tile_pool(name="work", bufs=4) as wp, \
             tc.tile_pool(name="psum", bufs=2, space="PSUM") as pp:
            # ---- persistent constants ----
            at_sb = [cp.tile([128, N], mmdt, tag=f"at{h}", name=f"at{h}") for h in range(2)]
            y_sb = [cp.tile([128, D], mmdt, tag=f"y{h}", name=f"y{h}") for h in range(2)]
            rnsx_sb = [cp.tile([128, D], F32, tag=f"rn{h}", name=f"rn{h}") for h in range(2)]
            wbd_sb = cp.tile([128, 128], mmdt, tag="wbd")
            wet_sb = cp.tile([D, D], F32, tag="wet")
            be2_sb = cp.tile([128, 1], F32, tag="be2")
            berep_sb = cp.tile([128, D], F32, tag="berep")
            id_sb = cp.tile([128, 128], F32, tag="ident")
            aggt_sb = cp.tile([D, N], F32, tag="aggt")
            besa_sb = [cp.tile([128, D], F32, tag=f"besa{h}", name=f"besa{h}") for h in range(2)]

            for h in range(2):
                nc.sync.dma_start(out=at_sb[h][:, :],
                                  in_=mm(at_d[128 * h:128 * h + 128, :]))
                nc.sync.dma_start(out=y_sb[h][:, :],
                                  in_=mm(y_d[128 * h:128 * h + 128, :]))
                nc.sync.dma_start(out=rnsx_sb[h][:, :],
                                  in_=rnsx_d[128 * h:128 * h + 128, :])
            nc.sync.dma_start(out=wbd_sb[:, :], in_=mm(wbd_d))
            nc.sync.dma_start(out=wet_sb[:, :], in_=wet_d)
            nc.sync.dma_start(out=be2_sb[:, :], in_=be2_d)
            nc.sync.dma_start(out=berep_sb[:, :], in_=berep_d)
            nc.sync.dma_start(out=id_sb[:, :], in_=ident_d)

            # ---- S_A = A @ node_x, then beSA = be * S_A  (for the bias
            # contribution to agg: sum_j A[i,j]*be[c]*Y[j,c]) ----
            for ib in range(2):
                sa_ps = pp.tile([128, D], F32, tag="pT", bufs=3)
                for h in range(2):
                    nc.tensor.matmul(sa_ps[:, :],
                                     at_sb[h][:, 128 * ib:128 * ib + 128],
                                     y_sb[h][:, :],
                                     start=(h == 0), stop=(h == 1))
                nc.vector.tensor_tensor(besa_sb[ib][:, :], sa_ps[:, :],
                                        berep_sb[:, :], mybir.AluOpType.mult)

            # ---- main loop over 32 groups of 8 i-rows ----
            for g in range(NG):
                i0 = 8 * g
                h_ps = pp.tile([D, 512], F32, tag="H", bufs=1)
                # one 512KB load per group; free layout (i, h, k) so the
                # HBM-side (i, h) dims merge into one 3-dim balanced AP
                emb2 = wp.tile([128, 1024], mmdt, tag="emb")
                emb2_r = emb2[:, :].rearrange("p (i h k) -> p i h k",
                                              i=8, h=2)
                for h in range(2):
                    nc.sync.dma_start(
                        out=emb2_r[:, :, h, :],
                        in_=mm(emb[i0:i0 + 8, 128 * h:128 * h + 128, :]
                               .rearrange("i j k -> j i k")))
                emb2_f = emb2[:, :].bitcast(F32).rearrange(
                    "p (i h k) -> p i h k", i=8, h=2)

                for h in range(2):
                    # mask: memb = emb * A^T[j', i]  (broadcast over k)
                    memb = wp.tile([128, 512], mmdt, tag="memb")
                    e_v = emb2_f[:, :, h, :]
                    a_v = at_sb[h][:, i0:i0 + 8].bitcast(F32).rearrange(
                        "p (i u) -> p i u", u=1)
                    e_b, a_b = broadcast_tensor_aps(e_v, a_v)
                    nc.gpsimd.tensor_tensor(
                        memb[:, :].bitcast(F32).rearrange(
                            "p (i k) -> p i k", i=8),
                        e_b, a_b, mybir.AluOpType.mult)
                    # H[c, (i, k)] += sum_j' Y[j', c] * memb[j', (i, k)]
                    nc.tensor.matmul(h_ps[:, :], y_sb[h][:, :], memb[:, :],
                                     start=(h == 0), stop=(h == 1))

                for q in range(2):
                    # forward transposes: [j', (h2, k)] -> [(h2, k), j'] per i
                    psumT = pp.tile([128, 512], F32, tag="pT", bufs=3)
                    for w in range(4):
                        isb = 4 * q + w
                        nc.tensor.matmul(psumT[:, 128 * w:128 * w + 128],
                                         emb2[:, 128 * isb:128 * isb + 128]
                                         .bitcast(F32),
                                         id_sb[:, :], is_transpose=True,
                                         start=True, stop=True)
                    embt = wp.tile([128, 512], mmdt, tag="embt")
                    nc.scalar.activation(embt[:, :], psumT[:, :],
                                         mybir.ActivationFunctionType.Copy)

                    # block-diag(We, We) main matmul -> edge_x^T in PSUM
                    e_ps = pp.tile([128, 512], F32, tag="pE")
                    nc.tensor.matmul(e_ps[:, :], wbd_sb[:, :], embt[:, :],
                                     start=True, stop=True)

                    # relu(+be) eviction (edge_out path, transposed layout)
                    eot = wp.tile([128, 512], F32, tag="eot")
                    nc.scalar.activation(eot[:, :], e_ps[:, :],
                                         mybir.ActivationFunctionType.Relu,
                                         bias=be2_sb[:, :])

                    # back transposes -> natural [j', (h, c)] per i
                    pnat = pp.tile([128, 512], F32, tag="pN")
                    for w in range(4):
                        nc.tensor.transpose(pnat[:, 128 * w:128 * w + 128],
                                            eot[:, 128 * w:128 * w + 128],
                                            id_sb[:, :].bitcast(F32))
                    eo = wp.tile([128, 512], F32, tag="eo")
                    nc.vector.tensor_copy(eo[:, :], pnat[:, :])
                    nc.scalar.dma_start(
                        out=edge_out[i0 + 4 * q:i0 + 4 * q + 4, :, :]
                            .rearrange("i (h j) c -> j i h c", h=2),
                        in_=eo[:, :].rearrange("p (i h c) -> p i h c",
                                               i=4, h=2))

                # agg: multiply H by We^T (bcast over i), reduce over k
                hw_sb = wp.tile([D, 512], F32, tag="hw")
                h_v = h_ps[:, :].rearrange("p (i k) -> p i k", i=8)
                w_v = wet_sb[:, :].rearrange("p (u k) -> p u k", u=1)
                h_b, w_b = broadcast_tensor_aps(h_v, w_v)
                nc.vector.tensor_tensor(
                    hw_sb[:, :].rearrange("p (i k) -> p i k", i=8),
                    h_b, w_b, mybir.AluOpType.mult)
                nc.vector.tensor_reduce(
                    aggt_sb[:, 8 * g:8 * g + 8],
                    hw_sb[:, :].rearrange("p (i k) -> p i k", i=8),
                    mybir.AxisListType.X, mybir.AluOpType.add)

            # ---- finalize node_out = relu(aggT^T + beSA) + relu_nsx ----
            for ib in range(2):
                agg_ps = pp.tile([128, D], F32, tag="pT", bufs=3)
                nc.tensor.transpose(agg_ps[:, :],
                                    aggt_sb[:, 128 * ib:128 * ib + 128],
                                    id_sb[0:D, 0:D].bitcast(F32))
                full = wp.tile([128, D], F32, tag="nfull")
                nc.vector.tensor_tensor(full[:, :], agg_ps[:, :],
                                        besa_sb[ib][:, :],
                                        mybir.AluOpType.add)
                rfull = wp.tile([128, D], F32, tag="nrelu")
                nc.scalar.activation(rfull[:, :], full[:, :],
                                     mybir.ActivationFunctionType.Relu)
                nsb = wp.tile([128, D], F32, tag="nout")
                nc.vector.tensor_tensor(nsb[:, :], rfull[:, :],
                                        rnsx_sb[ib][:, :],
                                        mybir.AluOpType.add)
                nc.sync.dma_start(out=node_out[128 * ib:128 * ib + 128, :],
                                  in_=nsb[:, :])

    nc.compile()
    return nc


def _host_prep(A, emb_node, emb_edge, Wn, bn, Wsn, bsn, We, be):
    """Per-core input maps. Heavy data (emb_edge) passes through untouched;
    only O(N*D)-scale tensors are laid out host-side."""
    A = np.asarray(A, np.float32)
    emb_node = np.asarray(emb_node, np.float32)
    emb_edge = np.ascontiguousarray(np.asarray(emb_edge, np.float32))
    Wn, bn = np.asarray(Wn, np.float32), np.asarray(bn, np.float32)
    Wsn, bsn = np.asarray(Wsn, np.float32), np.asarray(bsn, np.float32)
    We, be = np.asarray(We, np.float32), np.asarray(be, np.float32)

    wbd = np.zeros((128, 128), np.float32)
    wbd[:D, :D] = We
    wbd[D:, D:] = We
    wet = np.ascontiguousarray(We.T)
    be2 = np.concatenate([be, be])[:, None].astype(np.float32)
    berep = np.broadcast_to(be, (128, D)).astype(np.float32).copy()
    ident = np.eye(128, dtype=np.float32)

    in_maps = []
    for b in range(B):
        y = emb_node[b] @ Wn + bn
        rnsx = np.maximum(emb_node[b] @ Wsn + bsn, 0.0)
        in_maps.append({
            "emb": emb_edge[b],
            "at": np.ascontiguousarray(A[b].T),
            "y": np.ascontiguousarray(y.astype(np.float32)),
            "rnsx": np.ascontiguousarray(rnsx.astype(np.float32)),
            "wbd": wbd, "wet": wet, "be2": be2, "berep": berep,
            "ident": ident,
        })
    return in_maps


def _get_nc():
    key = ("nc", USE_F32R)
    if key not in _CACHE:
        _CACHE[key] = _build_nc(USE_F32R)
    return _CACHE[key]


def kernel(A, emb_node, emb_edge, Wn, bn, Wsn, bsn, We, be):
    in_maps = _host_prep(A, emb_node, emb_edge, Wn, bn, Wsn, bsn, We, be)
    nc = _get_nc()
    res = run_bass_kernel_spmd(nc, in_maps, core_ids=list(range(NCORES)))
    node = np.stack([res.results[b]["node_out"] for b in range(B)])
    edge = np.stack([res.results[b]["edge_out"] for b in range(B)])
    return node, edge
